# revision 41
# baseline (speedup 1.0000x reference)
"""Trainium2 Bass kernel for a dense transformer block (self-attn causal +
cross-attn + MLP), sharded over 8 NeuronCores without collectives.

Sharding: core c -> batch b = c//2, parity p = c%2. Each core computes the
output for query rows p::2 of batch b (1024 rows). K/V for self-attention are
recomputed per-core for the full 2048-row sequence. Parity is handled host-
side: for p=1 the sequence rows are pair-swapped so own tokens sit at even
positions for every core (one program for all 8 cores); the causal mask input
compensates for the within-pair key reordering.

v2: fp8e4m3 DoubleRow matmuls for all projections/PV/MLP, bf16 scores,
feature-major bf16 residual chain (SBUF-resident, no DRAM spills), causal mask
via PE identity-add of an additive mask into score PSUM, exp straight to fp8
with a -2 bias, softmax denominator reciprocal broadcast via SBUF->SBUF DMA,
LN2 computed feature-major with ones-matmul partition reductions. Power-of-2
scales keep fp8 operands out of the subnormal range; the scales unfold for
free inside the psum->sbuf bias copies.
"""
import sys

sys.path.insert(0, "/opt/trn_rl_repo")

import numpy as np
import ml_dtypes

import concourse.bass as bass
import concourse.tile as tile
from concourse import bacc, mybir
from concourse.bass_utils import run_bass_kernel_spmd
from concourse.masks import make_identity

F32 = mybir.dt.float32
BF16 = mybir.dt.bfloat16
FP8 = mybir.dt.float8e4
AF = mybir.ActivationFunctionType
OP = mybir.AluOpType
DR = mybir.MatmulPerfMode.DoubleRow

B, T, S, D = 4, 2048, 512, 768
NINP = 768
H, HD, HID = 12, 64, 3072
TQ = T // 2            # own query rows per core
DC = D // 128          # 6 feature chunks
HCN = HID // 128       # 24 hidden chunks
EPS = 1e-5

SW = 4096.0            # fp8 scale for most weights
SW2 = 8192.0           # fp8 scale for mw2
SE = 32.0              # fp8 scale for encoder activations
SX = 16.0              # fp8 scale for xn / x1 / h0 / y activations
ISXW = 1.0 / (SX * SW)
ISEW = 1.0 / (SE * SW)

_CACHE: dict = {}

f8 = ml_dtypes.float8_e4m3


def _bc(ap, n):
    """Partition-broadcast AP of a [1, n] slice."""
    return bass.AP(tensor=ap.tensor, offset=ap.offset, ap=[[0, 128], [1, n]])


def _build():
    nc = bacc.Bacc("TRN2", target_bir_lowering=False, debug=False)

    x_full = nc.dram_tensor("x_full", [T, D], BF16, kind="ExternalInput")
    wqp = nc.dram_tensor("wqp", [3 * 128, 2 * D], FP8, kind="ExternalInput")
    wkp = nc.dram_tensor("wkp", [3 * 128, 2 * D], FP8, kind="ExternalInput")
    wvp = nc.dram_tensor("wvp", [3 * 128, 2 * D], FP8, kind="ExternalInput")
    wop = nc.dram_tensor("wop", [3 * 128, 2 * D], FP8, kind="ExternalInput")
    cwqp = nc.dram_tensor("cwqp", [3 * 128, 2 * D], FP8, kind="ExternalInput")
    cwkp = nc.dram_tensor("cwkp", [4 * 128, 2 * D], FP8, kind="ExternalInput")
    cwvp = nc.dram_tensor("cwvp", [4 * 128, 2 * D], FP8, kind="ExternalInput")
    cwop = nc.dram_tensor("cwop", [3 * 128, 2 * D], FP8, kind="ExternalInput")
    mw1p = nc.dram_tensor("mw1p", [3 * 128, 2 * HID], FP8, kind="ExternalInput")
    mw2p = nc.dram_tensor("mw2p", [12 * 128, 2 * D], FP8, kind="ExternalInput")
    encp = nc.dram_tensor("encp", [4 * 128, 2 * S], FP8, kind="ExternalInput")
    bq = nc.dram_tensor("bq", [D], F32, kind="ExternalInput")
    bk = nc.dram_tensor("bk", [D], F32, kind="ExternalInput")
    bv = nc.dram_tensor("bv", [D], F32, kind="ExternalInput")
    cbq = nc.dram_tensor("cbq", [D], F32, kind="ExternalInput")
    cbk = nc.dram_tensor("cbk", [D], F32, kind="ExternalInput")
    cbv = nc.dram_tensor("cbv", [D], F32, kind="ExternalInput")
    mb1 = nc.dram_tensor("mb1", [HID], F32, kind="ExternalInput")
    mb2x = nc.dram_tensor("mb2x", [D], F32, kind="ExternalInput")  # mb2*SW2
    resg = nc.dram_tensor("resg", [D], F32, kind="ExternalInput")  # g1
    resb = nc.dram_tensor("resb", [D], F32, kind="ExternalInput")  # b1+bo+cbo
    maskq = nc.dram_tensor("maskq", [128, 1024], BF16, kind="ExternalInput")
    out_own = nc.dram_tensor("out_own", [TQ, D], BF16, kind="ExternalOutput")

    with tile.TileContext(nc) as tc:
        # pool stack; release order is the reverse of allocation order
        singles = tc.alloc_tile_pool(name="singles", bufs=1)
        pX2 = tc.alloc_tile_pool(name="pX2", bufs=1)       # to end
        w5pre = tc.alloc_tile_pool(name="w5pre", bufs=1)   # to end of ph5
        w4pre = tc.alloc_tile_pool(name="w4pre", bufs=1)   # to end of ph4
        pC = tc.alloc_tile_pool(name="pC", bufs=1)         # to end of ph4
        pX1 = tc.alloc_tile_pool(name="pX1", bufs=1)       # to end of ph4
        pQKV = tc.alloc_tile_pool(name="pQKV", bufs=1)     # to end of ph3
        pXN = tc.alloc_tile_pool(name="pXN", bufs=1)       # to end of ph3

        identf = singles.tile([128, 128], F32, name="identf")
        make_identity(nc, identf[:, :])
        identb = singles.tile([128, 128], BF16, name="identb")
        nc.vector.tensor_copy(identb, identf)
        eps_t = singles.tile([128, 1], F32, name="eps")
        nc.vector.memset(eps_t, EPS)
        neg2 = singles.tile([128, 1], F32, name="neg2")
        nc.vector.memset(neg2, -2.0)
        eps256 = singles.tile([1, 1], F32, name="eps256")
        nc.vector.memset(eps256, EPS / 256.0)
        ones1b = singles.tile([128, 1], BF16, name="ones1b")
        nc.vector.memset(ones1b, 1.0)
        onesrow = singles.tile([1, 128], BF16, name="onesrow")
        nc.vector.memset(onesrow, 1.0)
        mask_sb = singles.tile([128, 1024], BF16, name="mask_sb")
        nc.sync.dma_start(out=mask_sb, in_=maskq[:, :])

        def bias6(h, name, pool=None):
            n = h.shape[0]
            t = (pool or singles).tile([128, n // 128], F32, name=name)
            nc.sync.dma_start(out=t, in_=h.ap().rearrange("(c p) -> p c", p=128))
            return t

        def bias_bc(h, name, pool, n=D):
            t = pool.tile([128, n], F32, name=name)
            nc.gpsimd.dma_start(out=t, in_=_bc(h.ap(), n))
            return t

        bq6 = bias6(bq, "bq6")
        bk6 = bias6(bk, "bk6")
        cbq6 = bias6(cbq, "cbq6")
        cbk6 = bias6(cbk, "cbk6")
        g6 = bias6(resg, "g6")
        rb6 = bias6(resb, "rb6")
        mb2x6 = bias6(mb2x, "mb2x6")

        # prefetched weights (tiles here; DMAs issue at end of phase 1)
        mw1_sb = [w5pre.tile([128, 2, HID], FP8, name=f"mw1_{j}")
                  for j in range(3)]
        mb1c = w5pre.tile([128, HCN], F32, name="mb1c")
        cwq_sb = [w4pre.tile([128, 2, D], FP8, name=f"cwq{j}") for j in range(3)]
        cwk_sb = [w4pre.tile([128, 2, D], FP8, name=f"cwk{j}") for j in range(4)]
        cwv_sb = [w4pre.tile([128, 2, D], FP8, name=f"cwv{j}") for j in range(4)]
        cwo_sb = [w4pre.tile([128, 2, D], FP8, name=f"cwo{j}") for j in range(3)]
        enc_sb = [w4pre.tile([128, 2, S], FP8, name=f"enc{j}") for j in range(4)]

        # persistent activation tiles
        x2Tb = [pX2.tile([128, TQ], BF16, name=f"x2Tb{dc}") for dc in range(DC)]
        cqT = [pC.tile([128, TQ], BF16, name=f"cqT{dc}") for dc in range(DC)]
        ckT = [pC.tile([128, S], BF16, name=f"ckT{dc}") for dc in range(DC)]
        cvP = [pC.tile([128, 2, H, HD + 1], FP8, name=f"cvP{i}") for i in range(2)]
        x1Tb = [pX1.tile([128, TQ], BF16, name=f"x1Tb{dc}") for dc in range(DC)]
        x1T8 = [pX1.tile([128, 2, TQ], FP8, name=f"x1T8{j}") for j in range(3)]
        qT = [pQKV.tile([128, TQ], BF16, name=f"qT{dc}") for dc in range(DC)]
        kT = [pQKV.tile([128, T], BF16, name=f"kT{dc}") for dc in range(DC)]
        vP = [pQKV.tile([128, 2, H, HD + 1], FP8, name=f"vP{i}") for i in range(8)]
        xnT8 = [pXN.tile([128, 2, T], FP8, name=f"xnT8{j}") for j in range(3)]
        xnTb = [pXN.tile([128, TQ], BF16, name=f"xnTb{dc}") for dc in range(DC)]

        # ===== Phase 1: LN1 -> transposes -> Q/K/V projections ============
        with tc.tile_pool(name="w1", bufs=1) as w1, \
             tc.tile_pool(name="p1", bufs=4) as p1, \
             tc.tile_pool(name="p1s", bufs=6) as p1s, \
             tc.tile_pool(name="p1tp", bufs=1, space="PSUM") as p1tp, \
             tc.tile_pool(name="p1mm", bufs=2, space="PSUM") as p1mm:
            wq_sb = [w1.tile([128, 2, D], FP8, name=f"wq{j}") for j in range(3)]
            wk_sb = [w1.tile([128, 2, D], FP8, name=f"wk{j}") for j in range(3)]
            wv_sb = [w1.tile([128, 2, D], FP8, name=f"wv{j}") for j in range(3)]
            for j in range(3):
                nc.sync.dma_start(out=wq_sb[j], in_=wqp[j * 128:(j + 1) * 128, :])
                nc.sync.dma_start(out=wk_sb[j], in_=wkp[j * 128:(j + 1) * 128, :])
                nc.sync.dma_start(out=wv_sb[j], in_=wvp[j * 128:(j + 1) * 128, :])
            bv_bc = bias_bc(bv, "bv_bc", w1)
            for cp in range(8):
                nc.gpsimd.memset(vP[cp][:, :, :, HD:HD + 1], 1.0 / SX)

            for blk in range(4):  # 512-token blocks of the full sequence
                psT = [p1tp.tile([128, 2, 512], BF16, name=f"psT{j}")
                       for j in range(3)]
                for t4 in range(4):
                    tt = blk * 4 + t4
                    xt = p1.tile([128, D], BF16, name="xt")
                    nc.sync.dma_start(
                        out=xt, in_=x_full[tt * 128:(tt + 1) * 128, :])
                    xr = xt.rearrange("p (s f) -> p s f", f=256)
                    stats = p1s.tile([128, 3, 6], F32, name="bnst")
                    for si in range(3):
                        nc.vector.bn_stats(out=stats[:, si, :], in_=xr[:, si, :])
                    mv = p1s.tile([128, 2], F32, name="bnmv")
                    nc.vector.bn_aggr(out=mv, in_=stats)
                    std = p1s.tile([128, 1], F32, name="std")
                    nc.scalar.activation(std, mv[:, 1:2], AF.Sqrt, bias=eps_t)
                    rstd = p1s.tile([128, 1], F32, name="rstd")
                    nc.vector.reciprocal(rstd, std)
                    xnt = p1.tile([128, D], BF16, name="xnt")
                    nc.vector.tensor_scalar(xnt, xt, mv[:, 0:1], rstd,
                                            OP.subtract, OP.mult)
                    for dc in range(DC):
                        nc.tensor.transpose(
                            psT[dc // 2][:, dc % 2, t4 * 128:(t4 + 1) * 128],
                            xnt[:, dc * 128:(dc + 1) * 128], identb)
                # psum -> sbuf: fp8 (x SX) for matmuls; bf16 affine residual
                # (own = even columns after the host parity permutation)
                for j in range(3):
                    dst8 = xnT8[j][:, :, blk * 512:(blk + 1) * 512]
                    if j == 0:
                        nc.scalar.mul(dst8, psT[j], SX)
                    elif j == 1:
                        nc.gpsimd.tensor_scalar(dst8, psT[j], SX, None, OP.mult)
                    else:
                        nc.vector.tensor_scalar(dst8, psT[j], SX, None, OP.mult)
                for dc in range(DC):
                    nc.gpsimd.tensor_scalar(
                        xnTb[dc][:, blk * 256:(blk + 1) * 256],
                        psT[dc // 2][:, dc % 2, 0:512:2],
                        g6[:, dc:dc + 1], rb6[:, dc:dc + 1],
                        OP.mult, OP.add)
                # K projection for this block (bias copy on the idle Act)
                for dc in range(DC):
                    pp = p1mm.tile([128, 512], F32, name="kpp")
                    for half in range(2):
                        for j in range(3):
                            nc.tensor.matmul(
                                pp[:, half * 256:(half + 1) * 256],
                                wk_sb[j][:, :, dc * 128:(dc + 1) * 128],
                                xnT8[j][:, :, blk * 512 + half * 256:
                                        blk * 512 + (half + 1) * 256],
                                start=(j == 0), stop=(j == 2), perf_mode=DR)
                    nc.scalar.activation(
                        kT[dc][:, blk * 512:(blk + 1) * 512], pp,
                        AF.Identity, bias=bk6[:, dc:dc + 1], scale=ISXW)
                # V projection for this block
                for t4 in range(4):
                    tt = blk * 4 + t4
                    for hf in range(2):
                        pp = p1mm.tile([128, 384], F32, name="vpp")
                        for j in range(3):
                            nc.tensor.matmul(
                                pp,
                                xnT8[j][:, :, tt * 128:(tt + 1) * 128],
                                wv_sb[j][:, :, hf * 384:(hf + 1) * 384],
                                start=(j == 0), stop=(j == 2), perf_mode=DR)
                        eng = nc.vector if (t4 + hf) % 2 == 0 else nc.gpsimd
                        eng.scalar_tensor_tensor(
                            vP[tt // 2][:, tt % 2, hf * 6:(hf + 1) * 6, 0:HD],
                            pp, ISXW, bv_bc[:, hf * 384:(hf + 1) * 384],
                            OP.mult, OP.add)
            # Q projection (own = even columns, strided)
            for dc in range(DC):
                for qblk in range(2):
                    pp = p1mm.tile([128, 512], F32, name="kpp")
                    for half in range(2):
                        base = qblk * 1024 + half * 512
                        for j in range(3):
                            nc.tensor.matmul(
                                pp[:, half * 256:(half + 1) * 256],
                                wq_sb[j][:, :, dc * 128:(dc + 1) * 128],
                                xnT8[j][:, :, base:base + 512:2],
                                start=(j == 0), stop=(j == 2), perf_mode=DR)
                    nc.scalar.activation(
                        qT[dc][:, qblk * 512:(qblk + 1) * 512], pp,
                        AF.Identity, bias=bq6[:, dc:dc + 1], scale=ISXW)
            # prefetch phase-4/5 weights now; DMA is idle from here on
            for j in range(3):
                nc.sync.dma_start(out=cwq_sb[j], in_=cwqp[j * 128:(j + 1) * 128, :])
                nc.sync.dma_start(out=cwo_sb[j], in_=cwop[j * 128:(j + 1) * 128, :])
            for j in range(4):
                nc.sync.dma_start(out=cwk_sb[j], in_=cwkp[j * 128:(j + 1) * 128, :])
                nc.sync.dma_start(out=cwv_sb[j], in_=cwvp[j * 128:(j + 1) * 128, :])
                nc.sync.dma_start(out=enc_sb[j], in_=encp[j * 128:(j + 1) * 128, :])
            for j in range(3):
                nc.sync.dma_start(out=mw1_sb[j], in_=mw1p[j * 128:(j + 1) * 128, :])
            nc.sync.dma_start(out=mb1c,
                              in_=mb1.ap().rearrange("(c p) -> p c", p=128))

        # ===== Phase 3: causal self-attention =============================
        with tc.tile_pool(name="w3", bufs=1) as w3, \
             tc.tile_pool(name="y8p", bufs=2) as y8p, \
             tc.tile_pool(name="ytm3", bufs=2) as ytm3, \
             tc.tile_pool(name="pp3", bufs=4) as pp3, \
             tc.tile_pool(name="sps3", bufs=2, space="PSUM") as sps3, \
             tc.tile_pool(name="yps3", bufs=1, space="PSUM") as yps3, \
             tc.tile_pool(name="ptp3", bufs=1, space="PSUM") as ptp3, \
             tc.tile_pool(name="ops3", bufs=2, space="PSUM") as ops3:
            wo_sb = [w3.tile([128, 2, D], FP8, name=f"wo{j}") for j in range(3)]
            for j in range(3):
                nc.sync.dma_start(out=wo_sb[j], in_=wop[j * 128:(j + 1) * 128, :])
            def tail3(qb, yT8, ytm):
                # transpose y to feature-major fp8 pairs, O-proj, residual
                for qh in range(2):
                    ptT = ptp3.tile([128, D], BF16, name="ptT")
                    for dc in range(DC):
                        nc.tensor.transpose(
                            ptT[:, dc * 128:(dc + 1) * 128],
                            ytm[qh][:, dc * 128:(dc + 1) * 128], identb)
                    for j in range(3):
                        nc.vector.tensor_scalar(
                            yT8[j][:, :, qh * 128:(qh + 1) * 128],
                            ptT[:, j * 256:(j + 1) * 256], SX, None, OP.mult)
                for oc in range(DC):
                    xo = ops3.tile([128, 256], F32, name="xo")
                    for j in range(3):
                        nc.tensor.matmul(
                            xo, wo_sb[j][:, :, oc * 128:(oc + 1) * 128],
                            yT8[j], start=(j == 0), stop=(j == 2),
                            perf_mode=DR)
                    nc.vector.scalar_tensor_tensor(
                        x1Tb[oc][:, qb * 256:(qb + 1) * 256],
                        xo, ISXW, xnTb[oc][:, qb * 256:(qb + 1) * 256],
                        OP.mult, OP.add)
                    nc.gpsimd.tensor_scalar(
                        x1T8[oc // 2][:, oc % 2, qb * 256:(qb + 1) * 256],
                        x1Tb[oc][:, qb * 256:(qb + 1) * 256],
                        SX, None, OP.mult)

            pending = None
            for qb in range(4):
                ng = qb + 1
                yT8 = [y8p.tile([128, 2, 256], FP8, name=f"yT8{j}")
                       for j in range(3)]
                ytm = [ytm3.tile([128, D], BF16, name=f"ytm{qh}")
                       for qh in range(2)]
                for h in range(H):
                    if h == 2 and pending is not None:
                        pending()
                        pending = None
                    kb, ko = h // 2, (h % 2) * 64
                    y_ps = yps3.tile([128, 2, HD + 1], F32, name="yps")
                    for g in range(ng):
                        sps = sps3.tile([128, 4, 256], F32, name="sps")
                        diag = g == ng - 1
                        if diag:  # additive causal mask seeds the psum banks
                            for half in range(2):
                                nc.tensor.matmul(
                                    sps[:, half * 2:(half + 1) * 2, :],
                                    identb,
                                    mask_sb[:, half * 512:(half + 1) * 512],
                                    start=True, stop=False)
                        for c in range(4):
                            nc.tensor.matmul(
                                sps[:, c, :],
                                kT[kb][ko:ko + 64,
                                       (g * 4 + c) * 128:(g * 4 + c + 1) * 128],
                                qT[kb][ko:ko + 64, qb * 256:(qb + 1) * 256],
                                start=not diag, stop=(not diag) or (c % 2 == 1))
                        p_t = pp3.tile([128, 4, 256], FP8, name="P")
                        nc.scalar.activation(p_t, sps, AF.Exp, bias=neg2)
                        for qh in range(2):
                            for j2 in range(2):
                                nc.tensor.matmul(
                                    y_ps[:, qh, :],
                                    p_t[:, j2 * 2:(j2 + 1) * 2,
                                        qh * 128:(qh + 1) * 128],
                                    vP[g * 2 + j2][:, :, h, :],
                                    start=(g == 0 and j2 == 0),
                                    stop=(g == ng - 1 and j2 == 1),
                                    perf_mode=DR)
                    for qh in range(2):
                        with nc.allow_low_precision(reason="softmax denom"):
                            nc.gpsimd.tensor_scalar(
                                ytm[qh][:, h * HD:(h + 1) * HD],
                                y_ps[:, qh, 0:HD], y_ps[:, qh, HD:HD + 1],
                                None, OP.divide)
                pending = (lambda qb=qb, yT8=yT8, ytm=ytm:
                           tail3(qb, yT8, ytm))
            pending()
        pXN.release()
        pQKV.release()

        # ===== Phase 4: cross-attention ===================================
        with tc.tile_pool(name="w4", bufs=1) as w4, \
             tc.tile_pool(name="y4p", bufs=2) as y4p, \
             tc.tile_pool(name="ytm4", bufs=2) as ytm4, \
             tc.tile_pool(name="pp4", bufs=4) as pp4:
            cbv_bc = bias_bc(cbv, "cbv_bc", w4)
            for i in range(2):
                nc.gpsimd.memset(cvP[i][:, :, :, HD:HD + 1], 1.0 / SX)
            with tc.tile_pool(name="sps4", bufs=2, space="PSUM") as sps4, \
                 tc.tile_pool(name="yps4", bufs=1, space="PSUM") as yps4, \
                 tc.tile_pool(name="ptp4", bufs=1, space="PSUM") as ptp4, \
                 tc.tile_pool(name="prj4", bufs=2, space="PSUM") as prj4:
                for dc in range(DC):
                    for qblk in range(2):
                        pp = prj4.tile([128, 512], F32, name="prjp")
                        for half in range(2):
                            base = qblk * 512 + half * 256
                            for j in range(3):
                                nc.tensor.matmul(
                                    pp[:, half * 256:(half + 1) * 256],
                                    cwq_sb[j][:, :, dc * 128:(dc + 1) * 128],
                                    x1T8[j][:, :, base:base + 256],
                                    start=(j == 0), stop=(j == 2), perf_mode=DR)
                        nc.scalar.activation(
                            cqT[dc][:, qblk * 512:(qblk + 1) * 512], pp,
                            AF.Identity, bias=cbq6[:, dc:dc + 1], scale=ISXW)
                for dc in range(DC):
                    pp = prj4.tile([128, 512], F32, name="prjp")
                    for half in range(2):
                        for j in range(4):
                            nc.tensor.matmul(
                                pp[:, half * 256:(half + 1) * 256],
                                cwk_sb[j][:, :, dc * 128:(dc + 1) * 128],
                                enc_sb[j][:, :, half * 256:(half + 1) * 256],
                                start=(j == 0), stop=(j == 3), perf_mode=DR)
                    nc.scalar.activation(
                        ckT[dc], pp, AF.Identity,
                        bias=cbk6[:, dc:dc + 1], scale=ISEW)
                for st in range(4):
                    for hf in range(2):
                        ppw = prj4.tile([128, 512], F32, name="prjp")
                        pp = ppw[:, 0:384]
                        for j in range(4):
                            nc.tensor.matmul(
                                pp, enc_sb[j][:, :, st * 128:(st + 1) * 128],
                                cwv_sb[j][:, :, hf * 384:(hf + 1) * 384],
                                start=(j == 0), stop=(j == 3), perf_mode=DR)
                        nc.gpsimd.scalar_tensor_tensor(
                            cvP[st // 2][:, st % 2, hf * 6:(hf + 1) * 6, 0:HD],
                            pp, ISEW, cbv_bc[:, hf * 384:(hf + 1) * 384],
                            OP.mult, OP.add)

                def tail4(qb, yT8, ytm):
                    for qh in range(2):
                        ptT = ptp4.tile([128, D], BF16, name="ptTc")
                        for dc in range(DC):
                            nc.tensor.transpose(
                                ptT[:, dc * 128:(dc + 1) * 128],
                                ytm[qh][:, dc * 128:(dc + 1) * 128], identb)
                        for j in range(3):
                            nc.vector.tensor_scalar(
                                yT8[j][:, :, qh * 128:(qh + 1) * 128],
                                ptT[:, j * 256:(j + 1) * 256], SX, None,
                                OP.mult)
                    for oc in range(DC):
                        xow = prj4.tile([128, 512], F32, name="prjp")
                        xo = xow[:, 0:256]
                        for j in range(3):
                            nc.tensor.matmul(
                                xo, cwo_sb[j][:, :, oc * 128:(oc + 1) * 128],
                                yT8[j], start=(j == 0), stop=(j == 2),
                                perf_mode=DR)
                        nc.vector.scalar_tensor_tensor(
                            x2Tb[oc][:, qb * 256:(qb + 1) * 256],
                            xo, ISXW, x1Tb[oc][:, qb * 256:(qb + 1) * 256],
                            OP.mult, OP.add)

                pending = None
                for qb in range(4):
                    yT8 = [y4p.tile([128, 2, 256], FP8, name=f"yc8{j}")
                           for j in range(3)]
                    ytm = [ytm4.tile([128, D], BF16, name=f"ycm{qh}")
                           for qh in range(2)]
                    for h in range(H):
                        if h == 2 and pending is not None:
                            pending()
                            pending = None
                        kb, ko = h // 2, (h % 2) * 64
                        y_ps = yps4.tile([128, 2, HD + 1], F32, name="ypsc")
                        sps = sps4.tile([128, 4, 256], F32, name="spsc")
                        for c in range(4):
                            nc.tensor.matmul(
                                sps[:, c, :],
                                ckT[kb][ko:ko + 64, c * 128:(c + 1) * 128],
                                cqT[kb][ko:ko + 64, qb * 256:(qb + 1) * 256],
                                start=True, stop=True)
                        p_t = pp4.tile([128, 4, 256], FP8, name="Pc")
                        nc.scalar.activation(p_t, sps, AF.Exp, bias=neg2)
                        for qh in range(2):
                            for j2 in range(2):
                                nc.tensor.matmul(
                                    y_ps[:, qh, :],
                                    p_t[:, j2 * 2:(j2 + 1) * 2,
                                        qh * 128:(qh + 1) * 128],
                                    cvP[j2][:, :, h, :],
                                    start=(j2 == 0), stop=(j2 == 1),
                                    perf_mode=DR)
                        for qh in range(2):
                            with nc.allow_low_precision(reason="softmax denom"):
                                nc.gpsimd.tensor_scalar(
                                    ytm[qh][:, h * HD:(h + 1) * HD],
                                    y_ps[:, qh, 0:HD], y_ps[:, qh, HD:HD + 1],
                                    None, OP.divide)
                    pending = (lambda qb=qb, yT8=yT8, ytm=ytm:
                               tail4(qb, yT8, ytm))
                pending()
        pX1.release()
        pC.release()
        w4pre.release()

        # ===== Phase 5: LN2 (feature-major) + MLP + out ===================
        with tc.tile_pool(name="w5", bufs=1) as w5, \
             tc.tile_pool(name="p5a", bufs=1) as p5a, \
             tc.tile_pool(name="p5b", bufs=3) as p5b, \
             tc.tile_pool(name="h0p", bufs=1) as h0p, \
             tc.tile_pool(name="h1p", bufs=1) as h1p, \
             tc.tile_pool(name="oTp", bufs=1) as oTp:
            mw2_sb = [w5.tile([128, 2, D], FP8, name=f"mw2_{j}")
                      for j in range(12)]
            for j in range(12):
                nc.sync.dma_start(out=mw2_sb[j], in_=mw2p[j * 128:(j + 1) * 128, :])
            h0T8 = [h0p.tile([128, 2, TQ], FP8, name=f"h0T8{j}")
                    for j in range(3)]
            # LN2 stats via ones-matmul partition reduction
            with tc.tile_pool(name="p5st", bufs=1, space="PSUM") as p5st, \
                 tc.tile_pool(name="p5bc", bufs=1, space="PSUM") as p5bc:
                s1 = p5st.tile([1, TQ], F32, name="s1")
                s2 = p5st.tile([1, TQ], F32, name="s2")
                for blk2 in range(2):
                    sl = slice(blk2 * 512, (blk2 + 1) * 512)
                    for dc in range(DC):
                        nc.tensor.matmul(s1[0:1, sl], ones1b, x2Tb[dc][:, sl],
                                         start=(dc == 0), stop=(dc == DC - 1))
                    for dc in range(DC):
                        sq = p5b.tile([128, 512], BF16, name="sq")
                        nc.vector.tensor_mul(sq, x2Tb[dc][:, sl],
                                             x2Tb[dc][:, sl])
                        nc.tensor.matmul(s2[0:1, sl], ones1b, sq,
                                         start=(dc == 0), stop=(dc == DC - 1))
                mu_n = p5a.tile([1, TQ], F32, name="mu_n")
                nc.vector.tensor_scalar(mu_n, s1, -1.0 / D, None, OP.mult)
                msq = p5a.tile([1, TQ], F32, name="msq")
                nc.vector.tensor_scalar(msq, s2, 1.0 / D, None, OP.mult)
                mu2 = p5a.tile([1, TQ], F32, name="mu2")
                nc.vector.tensor_mul(mu2, mu_n, mu_n)
                var = p5a.tile([1, TQ], F32, name="var")
                nc.vector.tensor_sub(var, msq, mu2)
                # std16 = sqrt((var+eps)/256) = std/16 ; a = 1/std16 = 16*rstd
                std16 = p5a.tile([1, TQ], F32, name="std16")
                nc.scalar.activation(std16, var, AF.Sqrt, bias=eps256,
                                     scale=1.0 / 256.0)
                a_f = p5a.tile([1, TQ], F32, name="a_f")
                nc.vector.reciprocal(a_f, std16)
                a_b = p5a.tile([1, TQ], BF16, name="a_b")
                nc.vector.tensor_copy(a_b, a_f)
                c_b = p5a.tile([1, TQ], BF16, name="c_b")
                nc.vector.tensor_mul(c_b, mu_n, a_f)
                a_bc = p5bc.tile([128, TQ], F32, name="a_bc")
                c_bc = p5bc.tile([128, TQ], F32, name="c_bc")
                for blk2 in range(2):
                    sl = slice(blk2 * 512, (blk2 + 1) * 512)
                    nc.tensor.matmul(a_bc[:, sl], onesrow, a_b[0:1, sl],
                                     start=True, stop=True)
                    nc.tensor.matmul(c_bc[:, sl], onesrow, c_b[0:1, sl],
                                     start=True, stop=True)
                a_sb = p5a.tile([128, TQ], BF16, name="a_sb")
                nc.vector.tensor_copy(a_sb, a_bc)
                c_sb = p5a.tile([128, TQ], BF16, name="c_sb")
                nc.gpsimd.tensor_copy(c_sb, c_bc)
            for dc in range(DC):
                tmp = p5b.tile([128, TQ], BF16, name="h0tmp")
                nc.vector.tensor_mul(tmp, x2Tb[dc], a_sb)
                eng = nc.vector if dc % 2 == 0 else nc.gpsimd
                eng.tensor_tensor(
                    h0T8[dc // 2][:, dc % 2, :], tmp, c_sb, OP.add)
            # h1 = gelu((mw1^T h0 + mb1)) -> fp8
            h1T8 = [h1p.tile([128, 2, TQ], FP8, name=f"h1T8{j}")
                    for j in range(12)]
            x2r = [oTp.tile([128, D], BF16, name=f"x2r{tt}")
                   for tt in range(8)]
            mb2_bc = bias_bc(mb2x, "mb2_bc", w5)
            with tc.tile_pool(name="p5m1", bufs=2, space="PSUM") as p5m1, \
                 tc.tile_pool(name="p5m2", bufs=1, space="PSUM") as p5m2, \
                 tc.tile_pool(name="p5tp", bufs=1, space="PSUM") as p5tp, \
                 tc.tile_pool(name="p5o", bufs=3) as p5o:
                # x2 transpose-back to token-major (runs on idle PE during
                # gelu window); residual + mb2 pre-added (mb2_bc is mb2*SW2)
                for tt in range(8):
                    pt = p5tp.tile([128, D], BF16, name="x2tT")
                    for dc in range(DC):
                        nc.tensor.transpose(
                            pt[:, dc * 128:(dc + 1) * 128],
                            x2Tb[dc][:, tt * 128:(tt + 1) * 128], identb)
                    nc.gpsimd.scalar_tensor_tensor(
                        x2r[tt], mb2_bc, 1.0 / SW2, pt, OP.mult, OP.add)
                for hc in range(HCN):
                    pp = p5m1.tile([128, TQ], F32, name="h1pp")
                    for blk2 in range(2):
                        for half in range(2):
                            sl = slice(blk2 * 512 + half * 256,
                                       blk2 * 512 + (half + 1) * 256)
                            for j in range(3):
                                nc.tensor.matmul(
                                    pp[:, sl],
                                    mw1_sb[j][:, :, hc * 128:(hc + 1) * 128],
                                    h0T8[j][:, :, sl],
                                    start=(j == 0), stop=(j == 2),
                                    perf_mode=DR)
                    nc.scalar.activation(
                        h1T8[hc // 2][:, hc % 2, :], pp, AF.Gelu,
                        bias=mb1c[:, hc:hc + 1], scale=ISXW)
                # h2 token-major: out[tok, feat] interleaves with the gelus
                for tt in range(8):
                    pp = p5m2.tile([128, 4, 256], F32, name="h2pp")
                    for qf in range(4):
                        for j in range(12):
                            nc.tensor.matmul(
                                pp[:, qf, 0:192],
                                h1T8[j][:, :, tt * 128:(tt + 1) * 128],
                                mw2_sb[j][:, :, qf * 192:(qf + 1) * 192],
                                start=(j == 0), stop=(j == 11),
                                perf_mode=DR)
                    o_sb = p5o.tile([128, 4, 192], BF16, name="o_sb")
                    nc.vector.scalar_tensor_tensor(
                        o_sb, pp[:, :, 0:192], 1.0 / SW2,
                        x2r[tt].rearrange("p (a b) -> p a b", a=4),
                        OP.mult, OP.add)
                    nc.sync.dma_start(
                        out=out_own[tt * 128:(tt + 1) * 128, :],
                        in_=o_sb[:, :, :])
        w5pre.release()
        pX2.release()
        singles.release()

    nc.compile()
    return nc


def _get_nc():
    if "nc" not in _CACHE:
        _CACHE["nc"] = _build()
    return _CACHE["nc"]


def _pack2(w, scale):
    """[d_in, d_out] -> [d_in//256*128, 2*d_out] fp8 DoubleRow pair layout."""
    w = np.asarray(w, np.float32)
    d_in, d_out = w.shape
    nj = d_in // 256
    out = np.empty((nj * 128, 2 * d_out), np.float32)
    for j in range(nj):
        out[j * 128:(j + 1) * 128, :d_out] = w[j * 256:j * 256 + 128, :]
        out[j * 128:(j + 1) * 128, d_out:] = w[j * 256 + 128:j * 256 + 256, :]
    out = np.clip(out * scale, -224.0, 224.0)
    return np.ascontiguousarray(out.astype(f8))


def _make_in_maps(inputs):
    x = np.asarray(inputs["x"], np.float32)
    enc = np.asarray(inputs["encoder_hidden_states"], np.float32)
    scale = np.float32(1.0 / np.sqrt(HD))

    f32 = lambda a: np.ascontiguousarray(np.asarray(a, np.float32))
    g1 = np.asarray(inputs["ln1_g"], np.float64)
    b1 = np.asarray(inputs["ln1_b"], np.float64)
    g2 = np.asarray(inputs["ln2_g"], np.float64)
    b2 = np.asarray(inputs["ln2_b"], np.float64)
    sWq = np.asarray(inputs["sWq"], np.float64)
    sWk = np.asarray(inputs["sWk"], np.float64)
    sWv = np.asarray(inputs["sWv"], np.float64)
    mW1 = np.asarray(inputs["mW1"], np.float64)

    shared = dict(
        wqp=_pack2(g1[:, None] * sWq * scale, SW),
        bq=f32((b1 @ sWq + np.asarray(inputs["sbq"], np.float64)) * scale),
        wkp=_pack2(g1[:, None] * sWk, SW),
        bk=f32(b1 @ sWk + np.asarray(inputs["sbk"], np.float64)),
        wvp=_pack2(g1[:, None] * sWv, SW),
        bv=f32(b1 @ sWv + np.asarray(inputs["sbv"], np.float64)),
        wop=_pack2(np.asarray(inputs["sWo"]), SW),
        cwqp=_pack2(np.asarray(inputs["cWq"], np.float64) * scale, SW),
        cbq=f32(np.asarray(inputs["cbq"], np.float64) * scale),
        cwkp=_pack2(np.asarray(inputs["cWk"]), SW),
        cbk=f32(inputs["cbk"]),
        cwvp=_pack2(np.asarray(inputs["cWv"]), SW),
        cbv=f32(inputs["cbv"]),
        cwop=_pack2(np.asarray(inputs["cWo"]), SW),
        mw1p=_pack2(g2[:, None] * mW1, SW),
        mb1=f32(np.asarray(inputs["mb1"], np.float64) + b2 @ mW1),
        mw2p=_pack2(np.asarray(inputs["mW2"]), SW2),
        mb2x=f32(np.asarray(inputs["mb2"], np.float64) * SW2),
        resg=f32(inputs["ln1_g"]),
        resb=f32(b1 + np.asarray(inputs["sbo"], np.float64)
                 + np.asarray(inputs["cbo"], np.float64)),
    )
    kk = np.arange(128)[:, None]
    jq = np.arange(1024)[None, :]
    in_maps = []
    for c in range(8):
        b, p = c // 2, c % 2
        m = dict(shared)
        xb = x[b]
        if p == 1:
            xb = xb.reshape(T // 2, 2, D)[:, ::-1, :].reshape(T, D)
        m["x_full"] = np.ascontiguousarray(xb.astype(ml_dtypes.bfloat16))
        m["encp"] = _pack2(enc[b].T, SE)
        # key row k of a 128-chunk holds global key 128*j + kg(k)
        if p == 0:
            kg = kk
        else:
            kg = kk + 1 - 2 * (kk % 2)
        valid = (2 * (jq % 256) + p) >= (128 * (jq // 256) + kg)
        m["maskq"] = np.ascontiguousarray(
            np.where(valid, 0.0, -30000.0).astype(ml_dtypes.bfloat16))
        in_maps.append(m)
    return in_maps


def kernel(**inputs):
    in_maps = _make_in_maps(inputs)
    nc = _get_nc()
    res = run_bass_kernel_spmd(nc, in_maps, core_ids=list(range(8)))
    out = np.empty((B, T, NINP), np.float32)
    for c in range(8):
        b, p = c // 2, c % 2
        out[b, p::2] = np.asarray(res.results[c]["out_own"], np.float32)
    return out


# revision 52
# speedup vs baseline: 1.0576x; 1.0576x over previous
"""Trainium2 Bass kernel for a dense transformer block (self-attn causal +
cross-attn + MLP), sharded over 8 NeuronCores without collectives.

Sharding: core c -> batch b = c//2, parity p = c%2. Each core computes the
output for query rows p::2 of batch b (1024 rows). K/V for self-attention are
recomputed per-core for the full 2048-row sequence. Parity is handled host-
side: for p=1 the sequence rows are pair-swapped so own tokens sit at even
positions for every core (one program for all 8 cores); the causal mask input
compensates for the within-pair key reordering.

v2: fp8e4m3 DoubleRow matmuls for all projections/PV/MLP, bf16 scores,
feature-major bf16 residual chain (SBUF-resident, no DRAM spills), causal mask
via PE identity-add of an additive mask into score PSUM, exp straight to fp8
with a -2 bias, softmax denominator reciprocal broadcast via SBUF->SBUF DMA,
LN2 computed feature-major with ones-matmul partition reductions. Power-of-2
scales keep fp8 operands out of the subnormal range; the scales unfold for
free inside the psum->sbuf bias copies.
"""
import sys

sys.path.insert(0, "/opt/trn_rl_repo")

import numpy as np
import ml_dtypes

import concourse.bass as bass
import concourse.tile as tile
from concourse import bacc, mybir
from concourse.bass_utils import run_bass_kernel_spmd
from concourse.masks import make_identity

F32 = mybir.dt.float32
BF16 = mybir.dt.bfloat16
FP8 = mybir.dt.float8e4
AF = mybir.ActivationFunctionType
OP = mybir.AluOpType
DR = mybir.MatmulPerfMode.DoubleRow

B, T, S, D = 4, 2048, 512, 768
NINP = 768
H, HD, HID = 12, 64, 3072
TQ = T // 2            # own query rows per core
DC = D // 128          # 6 feature chunks
HCN = HID // 128       # 24 hidden chunks
EPS = 1e-5

SW = 4096.0            # fp8 scale for most weights
SW2 = 8192.0           # fp8 scale for mw2
SE = 32.0              # fp8 scale for encoder activations
SX = 16.0              # fp8 scale for xn / x1 / h0 / y activations
ISXW = 1.0 / (SX * SW)
ISEW = 1.0 / (SE * SW)

_CACHE: dict = {}

f8 = ml_dtypes.float8_e4m3


def _bc(ap, n):
    """Partition-broadcast AP of a [1, n] slice."""
    return bass.AP(tensor=ap.tensor, offset=ap.offset, ap=[[0, 128], [1, n]])


def _build():
    nc = bacc.Bacc("TRN2", target_bir_lowering=False, debug=False)

    x_full = nc.dram_tensor("x_full", [T, D], BF16, kind="ExternalInput")
    wqp = nc.dram_tensor("wqp", [3 * 128, 2 * D], FP8, kind="ExternalInput")
    wkp = nc.dram_tensor("wkp", [3 * 128, 2 * D], FP8, kind="ExternalInput")
    wvp = nc.dram_tensor("wvp", [3 * 128, 2 * D], FP8, kind="ExternalInput")
    wop = nc.dram_tensor("wop", [3 * 128, 2 * D], FP8, kind="ExternalInput")
    cwqp = nc.dram_tensor("cwqp", [3 * 128, 2 * D], FP8, kind="ExternalInput")
    cwkp = nc.dram_tensor("cwkp", [4 * 128, 2 * D], FP8, kind="ExternalInput")
    cwvp = nc.dram_tensor("cwvp", [4 * 128, 2 * D], FP8, kind="ExternalInput")
    cwop = nc.dram_tensor("cwop", [3 * 128, 2 * D], FP8, kind="ExternalInput")
    mw1p = nc.dram_tensor("mw1p", [3 * 128, 2 * HID], FP8, kind="ExternalInput")
    mw2p = nc.dram_tensor("mw2p", [12 * 128, 2 * D], FP8, kind="ExternalInput")
    encp = nc.dram_tensor("encp", [4 * 128, 2 * S], FP8, kind="ExternalInput")
    bq = nc.dram_tensor("bq", [D], F32, kind="ExternalInput")
    bk = nc.dram_tensor("bk", [D], F32, kind="ExternalInput")
    bv = nc.dram_tensor("bv", [D], F32, kind="ExternalInput")
    cbq = nc.dram_tensor("cbq", [D], F32, kind="ExternalInput")
    cbk = nc.dram_tensor("cbk", [D], F32, kind="ExternalInput")
    cbv = nc.dram_tensor("cbv", [D], F32, kind="ExternalInput")
    mb1 = nc.dram_tensor("mb1", [HID], F32, kind="ExternalInput")
    mb2x = nc.dram_tensor("mb2x", [D], F32, kind="ExternalInput")  # mb2*SW2
    resg = nc.dram_tensor("resg", [D], F32, kind="ExternalInput")  # g1
    resb = nc.dram_tensor("resb", [D], F32, kind="ExternalInput")  # b1+bo+cbo
    maskq = nc.dram_tensor("maskq", [128, 1024], BF16, kind="ExternalInput")
    out_own = nc.dram_tensor("out_own", [TQ, D], BF16, kind="ExternalOutput")

    with tile.TileContext(nc) as tc:
        # pool stack; release order is the reverse of allocation order
        singles = tc.alloc_tile_pool(name="singles", bufs=1)
        pX2 = tc.alloc_tile_pool(name="pX2", bufs=1)       # to end
        w4pre = tc.alloc_tile_pool(name="w4pre", bufs=1)   # to end of ph4
        pC = tc.alloc_tile_pool(name="pC", bufs=1)         # to end of ph4
        pX1 = tc.alloc_tile_pool(name="pX1", bufs=1)       # to end of ph4
        pQKV = tc.alloc_tile_pool(name="pQKV", bufs=1)     # to end of ph3
        pXN = tc.alloc_tile_pool(name="pXN", bufs=1)       # to end of ph3

        identf = singles.tile([128, 128], F32, name="identf")
        make_identity(nc, identf[:, :])
        identb = singles.tile([128, 128], BF16, name="identb")
        nc.vector.tensor_copy(identb, identf)
        eps_t = singles.tile([128, 1], F32, name="eps")
        nc.vector.memset(eps_t, EPS)
        neg2 = singles.tile([128, 1], F32, name="neg2")
        nc.vector.memset(neg2, -2.0)
        eps256 = singles.tile([1, 1], F32, name="eps256")
        nc.vector.memset(eps256, EPS / 256.0)
        ones1b = singles.tile([128, 1], BF16, name="ones1b")
        nc.vector.memset(ones1b, 1.0)
        onesrow = singles.tile([1, 128], BF16, name="onesrow")
        nc.vector.memset(onesrow, 1.0)
        mask_sb = singles.tile([128, 1024], BF16, name="mask_sb")
        nc.sync.dma_start(out=mask_sb, in_=maskq[:, :])

        def bias6(h, name, pool=None):
            n = h.shape[0]
            t = (pool or singles).tile([128, n // 128], F32, name=name)
            nc.sync.dma_start(out=t, in_=h.ap().rearrange("(c p) -> p c", p=128))
            return t

        def bias_bc(h, name, pool, n=D):
            t = pool.tile([128, n], F32, name=name)
            nc.gpsimd.dma_start(out=t, in_=_bc(h.ap(), n))
            return t

        bq6 = bias6(bq, "bq6")
        bk6 = bias6(bk, "bk6")
        cbq6 = bias6(cbq, "cbq6")
        cbk6 = bias6(cbk, "cbk6")
        g6 = bias6(resg, "g6")
        rb6 = bias6(resb, "rb6")
        mb2x6 = bias6(mb2x, "mb2x6")

        # prefetched weights (tiles here; DMAs issue at end of phase 1)
        cwq_sb = [w4pre.tile([128, 2, D], FP8, name=f"cwq{j}") for j in range(3)]
        cwk_sb = [w4pre.tile([128, 2, D], FP8, name=f"cwk{j}") for j in range(4)]
        cwv_sb = [w4pre.tile([128, 2, D], FP8, name=f"cwv{j}") for j in range(4)]
        cwo_sb = [w4pre.tile([128, 2, D], FP8, name=f"cwo{j}") for j in range(3)]
        enc_sb = [w4pre.tile([128, 2, S], FP8, name=f"enc{j}") for j in range(4)]

        # persistent activation tiles
        x2Tb = [pX2.tile([128, TQ], BF16, name=f"x2Tb{dc}") for dc in range(DC)]
        x2sq = [pX2.tile([128, TQ], BF16, name=f"x2sq{dc}") for dc in range(DC)]
        cqT = [pC.tile([128, TQ], BF16, name=f"cqT{dc}") for dc in range(DC)]
        ckT = [pC.tile([128, S], BF16, name=f"ckT{dc}") for dc in range(DC)]
        cvP = [pC.tile([128, 2, H, HD + 1], FP8, name=f"cvP{i}") for i in range(2)]
        x1Tb = [pX1.tile([128, TQ], BF16, name=f"x1Tb{dc}") for dc in range(DC)]
        x1T8 = [pX1.tile([128, 2, TQ], FP8, name=f"x1T8{j}") for j in range(3)]
        qT = [pQKV.tile([128, TQ], BF16, name=f"qT{dc}") for dc in range(DC)]
        kT = [pQKV.tile([128, T], BF16, name=f"kT{dc}") for dc in range(DC)]
        vP = [pQKV.tile([128, 2, H, HD + 1], FP8, name=f"vP{i}") for i in range(8)]
        xnT8 = [pXN.tile([128, 2, T], FP8, name=f"xnT8{j}") for j in range(3)]
        xnTb = [pXN.tile([128, TQ], BF16, name=f"xnTb{dc}") for dc in range(DC)]

        # ===== Phase 1: LN1 -> transposes -> Q/K/V projections ============
        with tc.tile_pool(name="w1", bufs=1) as w1, \
             tc.tile_pool(name="p1", bufs=4) as p1, \
             tc.tile_pool(name="p1s", bufs=6) as p1s, \
             tc.tile_pool(name="p1tp", bufs=1, space="PSUM") as p1tp, \
             tc.tile_pool(name="p1mm", bufs=2, space="PSUM") as p1mm:
            wq_sb = [w1.tile([128, 2, D], FP8, name=f"wq{j}") for j in range(3)]
            wk_sb = [w1.tile([128, 2, D], FP8, name=f"wk{j}") for j in range(3)]
            wv_sb = [w1.tile([128, 2, D], FP8, name=f"wv{j}") for j in range(3)]
            for j in range(3):
                nc.sync.dma_start(out=wq_sb[j], in_=wqp[j * 128:(j + 1) * 128, :])
                nc.sync.dma_start(out=wk_sb[j], in_=wkp[j * 128:(j + 1) * 128, :])
                nc.sync.dma_start(out=wv_sb[j], in_=wvp[j * 128:(j + 1) * 128, :])
            bv_bc = bias_bc(bv, "bv_bc", w1)
            for cp in range(8):
                nc.gpsimd.memset(vP[cp][:, :, :, HD:HD + 1], 1.0 / SX)

            for blk in range(4):  # 512-token blocks of the full sequence
                psT = [p1tp.tile([128, 2, 512], BF16, name=f"psT{j}")
                       for j in range(3)]
                for t4 in range(4):
                    tt = blk * 4 + t4
                    xt = p1.tile([128, D], BF16, name="xt")
                    nc.sync.dma_start(
                        out=xt, in_=x_full[tt * 128:(tt + 1) * 128, :])
                    xr = xt.rearrange("p (s f) -> p s f", f=256)
                    stats = p1s.tile([128, 3, 6], F32, name="bnst")
                    for si in range(3):
                        nc.vector.bn_stats(out=stats[:, si, :], in_=xr[:, si, :])
                    mv = p1s.tile([128, 2], F32, name="bnmv")
                    nc.vector.bn_aggr(out=mv, in_=stats)
                    std = p1s.tile([128, 1], F32, name="std")
                    nc.scalar.activation(std, mv[:, 1:2], AF.Sqrt, bias=eps_t)
                    rstd = p1s.tile([128, 1], F32, name="rstd")
                    nc.vector.reciprocal(rstd, std)
                    xnt = p1.tile([128, D], BF16, name="xnt")
                    nc.vector.tensor_scalar(xnt, xt, mv[:, 0:1], rstd,
                                            OP.subtract, OP.mult)
                    for dc in range(DC):
                        nc.tensor.transpose(
                            psT[dc // 2][:, dc % 2, t4 * 128:(t4 + 1) * 128],
                            xnt[:, dc * 128:(dc + 1) * 128], identb)
                # psum -> sbuf: fp8 (x SX) for matmuls; bf16 affine residual
                # (own = even columns after the host parity permutation)
                for j in range(3):
                    dst8 = xnT8[j][:, :, blk * 512:(blk + 1) * 512]
                    if j == 0:
                        nc.scalar.mul(dst8, psT[j], SX)
                    elif j == 1:
                        nc.gpsimd.tensor_scalar(dst8, psT[j], SX, None, OP.mult)
                    else:
                        nc.vector.tensor_scalar(dst8, psT[j], SX, None, OP.mult)
                for dc in range(DC):
                    nc.gpsimd.tensor_scalar(
                        xnTb[dc][:, blk * 256:(blk + 1) * 256],
                        psT[dc // 2][:, dc % 2, 0:512:2],
                        g6[:, dc:dc + 1], rb6[:, dc:dc + 1],
                        OP.mult, OP.add)
                # K projection for this block (bias copy on the idle Act)
                for dc in range(DC):
                    pp = p1mm.tile([128, 512], F32, name="kpp")
                    for half in range(2):
                        for j in range(3):
                            nc.tensor.matmul(
                                pp[:, half * 256:(half + 1) * 256],
                                wk_sb[j][:, :, dc * 128:(dc + 1) * 128],
                                xnT8[j][:, :, blk * 512 + half * 256:
                                        blk * 512 + (half + 1) * 256],
                                start=(j == 0), stop=(j == 2), perf_mode=DR)
                    nc.scalar.activation(
                        kT[dc][:, blk * 512:(blk + 1) * 512], pp,
                        AF.Identity, bias=bk6[:, dc:dc + 1], scale=ISXW)
                # V projection for this block
                for t4 in range(4):
                    tt = blk * 4 + t4
                    for hf in range(2):
                        pp = p1mm.tile([128, 384], F32, name="vpp")
                        for j in range(3):
                            nc.tensor.matmul(
                                pp,
                                xnT8[j][:, :, tt * 128:(tt + 1) * 128],
                                wv_sb[j][:, :, hf * 384:(hf + 1) * 384],
                                start=(j == 0), stop=(j == 2), perf_mode=DR)
                        eng = nc.vector if (t4 + hf) % 2 == 0 else nc.gpsimd
                        eng.scalar_tensor_tensor(
                            vP[tt // 2][:, tt % 2, hf * 6:(hf + 1) * 6, 0:HD],
                            pp, ISXW, bv_bc[:, hf * 384:(hf + 1) * 384],
                            OP.mult, OP.add)
            # Q projection (own = even columns, strided)
            for dc in range(DC):
                for qblk in range(2):
                    pp = p1mm.tile([128, 512], F32, name="kpp")
                    for half in range(2):
                        base = qblk * 1024 + half * 512
                        for j in range(3):
                            nc.tensor.matmul(
                                pp[:, half * 256:(half + 1) * 256],
                                wq_sb[j][:, :, dc * 128:(dc + 1) * 128],
                                xnT8[j][:, :, base:base + 512:2],
                                start=(j == 0), stop=(j == 2), perf_mode=DR)
                    nc.scalar.activation(
                        qT[dc][:, qblk * 512:(qblk + 1) * 512], pp,
                        AF.Identity, bias=bq6[:, dc:dc + 1], scale=ISXW)
            # prefetch phase-4/5 weights now; DMA is idle from here on
            for j in range(3):
                nc.sync.dma_start(out=cwq_sb[j], in_=cwqp[j * 128:(j + 1) * 128, :])
                nc.sync.dma_start(out=cwo_sb[j], in_=cwop[j * 128:(j + 1) * 128, :])
            for j in range(4):
                nc.sync.dma_start(out=cwk_sb[j], in_=cwkp[j * 128:(j + 1) * 128, :])
                nc.sync.dma_start(out=cwv_sb[j], in_=cwvp[j * 128:(j + 1) * 128, :])
                nc.sync.dma_start(out=enc_sb[j], in_=encp[j * 128:(j + 1) * 128, :])


        # ===== Phase 3: causal self-attention =============================
        with tc.tile_pool(name="w3", bufs=1) as w3, \
             tc.tile_pool(name="y8p", bufs=2) as y8p, \
             tc.tile_pool(name="ytm3", bufs=2) as ytm3, \
             tc.tile_pool(name="pp3", bufs=4) as pp3, \
             tc.tile_pool(name="sps3", bufs=2, space="PSUM") as sps3, \
             tc.tile_pool(name="yps3", bufs=1, space="PSUM") as yps3, \
             tc.tile_pool(name="ptp3", bufs=1, space="PSUM") as ptp3, \
             tc.tile_pool(name="ops3", bufs=2, space="PSUM") as ops3:
            wo_sb = [w3.tile([128, 2, D], FP8, name=f"wo{j}") for j in range(3)]
            for j in range(3):
                nc.sync.dma_start(out=wo_sb[j], in_=wop[j * 128:(j + 1) * 128, :])
            def tail3(qb, yT8, ytm):
                # transpose y to feature-major fp8 pairs, O-proj, residual
                for qh in range(2):
                    ptT = ptp3.tile([128, D], BF16, name="ptT")
                    for dc in range(DC):
                        nc.tensor.transpose(
                            ptT[:, dc * 128:(dc + 1) * 128],
                            ytm[qh][:, dc * 128:(dc + 1) * 128], identb)
                    for j in range(3):
                        nc.vector.tensor_scalar(
                            yT8[j][:, :, qh * 128:(qh + 1) * 128],
                            ptT[:, j * 256:(j + 1) * 256], SX, None, OP.mult)
                for oc in range(DC):
                    xo = ops3.tile([128, 256], F32, name="xo")
                    for j in range(3):
                        nc.tensor.matmul(
                            xo, wo_sb[j][:, :, oc * 128:(oc + 1) * 128],
                            yT8[j], start=(j == 0), stop=(j == 2),
                            perf_mode=DR)
                    nc.vector.scalar_tensor_tensor(
                        x1Tb[oc][:, qb * 256:(qb + 1) * 256],
                        xo, ISXW, xnTb[oc][:, qb * 256:(qb + 1) * 256],
                        OP.mult, OP.add)
                    nc.gpsimd.tensor_scalar(
                        x1T8[oc // 2][:, oc % 2, qb * 256:(qb + 1) * 256],
                        x1Tb[oc][:, qb * 256:(qb + 1) * 256],
                        SX, None, OP.mult)

            pending = None
            for qb in range(4):
                ng = qb + 1
                yT8 = [y8p.tile([128, 2, 256], FP8, name=f"yT8{j}")
                       for j in range(3)]
                ytm = [ytm3.tile([128, D], BF16, name=f"ytm{qh}")
                       for qh in range(2)]
                for h in range(H):
                    if h == 2 and pending is not None:
                        pending()
                        pending = None
                    kb, ko = h // 2, (h % 2) * 64
                    y_ps = yps3.tile([128, 2, HD + 1], F32, name="yps")
                    for g in range(ng):
                        sps = sps3.tile([128, 4, 256], F32, name="sps")
                        diag = g == ng - 1
                        if diag:  # additive causal mask seeds the psum banks
                            for half in range(2):
                                nc.tensor.matmul(
                                    sps[:, half * 2:(half + 1) * 2, :],
                                    identb,
                                    mask_sb[:, half * 512:(half + 1) * 512],
                                    start=True, stop=False)
                        for c in range(4):
                            nc.tensor.matmul(
                                sps[:, c, :],
                                kT[kb][ko:ko + 64,
                                       (g * 4 + c) * 128:(g * 4 + c + 1) * 128],
                                qT[kb][ko:ko + 64, qb * 256:(qb + 1) * 256],
                                start=not diag, stop=(not diag) or (c % 2 == 1))
                        p_t = pp3.tile([128, 4, 256], FP8, name="P")
                        nc.scalar.activation(p_t, sps, AF.Exp, bias=neg2)
                        for qh in range(2):
                            for j2 in range(2):
                                nc.tensor.matmul(
                                    y_ps[:, qh, :],
                                    p_t[:, j2 * 2:(j2 + 1) * 2,
                                        qh * 128:(qh + 1) * 128],
                                    vP[g * 2 + j2][:, :, h, :],
                                    start=(g == 0 and j2 == 0),
                                    stop=(g == ng - 1 and j2 == 1),
                                    perf_mode=DR)
                    for qh in range(2):
                        with nc.allow_low_precision(reason="softmax denom"):
                            nc.gpsimd.tensor_scalar(
                                ytm[qh][:, h * HD:(h + 1) * HD],
                                y_ps[:, qh, 0:HD], y_ps[:, qh, HD:HD + 1],
                                None, OP.divide)
                pending = (lambda qb=qb, yT8=yT8, ytm=ytm:
                           tail3(qb, yT8, ytm))
            pending()
        pXN.release()
        pQKV.release()

        # ===== Phase 4: cross-attention ===================================
        with tc.tile_pool(name="w4", bufs=1) as w4, \
             tc.tile_pool(name="y4p", bufs=2) as y4p, \
             tc.tile_pool(name="ytm4", bufs=2) as ytm4, \
             tc.tile_pool(name="pp4", bufs=4) as pp4:
            cbv_bc = bias_bc(cbv, "cbv_bc", w4)
            for i in range(2):
                nc.gpsimd.memset(cvP[i][:, :, :, HD:HD + 1], 1.0 / SX)
            with tc.tile_pool(name="sps4", bufs=2, space="PSUM") as sps4, \
                 tc.tile_pool(name="yps4", bufs=1, space="PSUM") as yps4, \
                 tc.tile_pool(name="ptp4", bufs=1, space="PSUM") as ptp4, \
                 tc.tile_pool(name="prj4", bufs=2, space="PSUM") as prj4:
                for dc in range(DC):
                    for qblk in range(2):
                        pp = prj4.tile([128, 512], F32, name="prjp")
                        for half in range(2):
                            base = qblk * 512 + half * 256
                            for j in range(3):
                                nc.tensor.matmul(
                                    pp[:, half * 256:(half + 1) * 256],
                                    cwq_sb[j][:, :, dc * 128:(dc + 1) * 128],
                                    x1T8[j][:, :, base:base + 256],
                                    start=(j == 0), stop=(j == 2), perf_mode=DR)
                        nc.scalar.activation(
                            cqT[dc][:, qblk * 512:(qblk + 1) * 512], pp,
                            AF.Identity, bias=cbq6[:, dc:dc + 1], scale=ISXW)
                for dc in range(DC):
                    pp = prj4.tile([128, 512], F32, name="prjp")
                    for half in range(2):
                        for j in range(4):
                            nc.tensor.matmul(
                                pp[:, half * 256:(half + 1) * 256],
                                cwk_sb[j][:, :, dc * 128:(dc + 1) * 128],
                                enc_sb[j][:, :, half * 256:(half + 1) * 256],
                                start=(j == 0), stop=(j == 3), perf_mode=DR)
                    nc.scalar.activation(
                        ckT[dc], pp, AF.Identity,
                        bias=cbk6[:, dc:dc + 1], scale=ISEW)
                for st in range(4):
                    for hf in range(2):
                        ppw = prj4.tile([128, 512], F32, name="prjp")
                        pp = ppw[:, 0:384]
                        for j in range(4):
                            nc.tensor.matmul(
                                pp, enc_sb[j][:, :, st * 128:(st + 1) * 128],
                                cwv_sb[j][:, :, hf * 384:(hf + 1) * 384],
                                start=(j == 0), stop=(j == 3), perf_mode=DR)
                        nc.gpsimd.scalar_tensor_tensor(
                            cvP[st // 2][:, st % 2, hf * 6:(hf + 1) * 6, 0:HD],
                            pp, ISEW, cbv_bc[:, hf * 384:(hf + 1) * 384],
                            OP.mult, OP.add)

                def tail4(qb, yT8, ytm):
                    for qh in range(2):
                        ptT = ptp4.tile([128, D], BF16, name="ptTc")
                        for dc in range(DC):
                            nc.tensor.transpose(
                                ptT[:, dc * 128:(dc + 1) * 128],
                                ytm[qh][:, dc * 128:(dc + 1) * 128], identb)
                        for j in range(3):
                            nc.vector.tensor_scalar(
                                yT8[j][:, :, qh * 128:(qh + 1) * 128],
                                ptT[:, j * 256:(j + 1) * 256], SX, None,
                                OP.mult)
                    for oc in range(DC):
                        xow = prj4.tile([128, 512], F32, name="prjp")
                        xo = xow[:, 0:256]
                        for j in range(3):
                            nc.tensor.matmul(
                                xo, cwo_sb[j][:, :, oc * 128:(oc + 1) * 128],
                                yT8[j], start=(j == 0), stop=(j == 2),
                                perf_mode=DR)
                        nc.vector.scalar_tensor_tensor(
                            x2Tb[oc][:, qb * 256:(qb + 1) * 256],
                            xo, ISXW, x1Tb[oc][:, qb * 256:(qb + 1) * 256],
                            OP.mult, OP.add)
                        eng = nc.gpsimd if oc % 2 else nc.vector
                        eng.tensor_mul(
                            x2sq[oc][:, qb * 256:(qb + 1) * 256],
                            x2Tb[oc][:, qb * 256:(qb + 1) * 256],
                            x2Tb[oc][:, qb * 256:(qb + 1) * 256])

                pending = None
                for qb in range(4):
                    yT8 = [y4p.tile([128, 2, 256], FP8, name=f"yc8{j}")
                           for j in range(3)]
                    ytm = [ytm4.tile([128, D], BF16, name=f"ycm{qh}")
                           for qh in range(2)]
                    for h in range(H):
                        if h == 2 and pending is not None:
                            pending()
                            pending = None
                        kb, ko = h // 2, (h % 2) * 64
                        y_ps = yps4.tile([128, 2, HD + 1], F32, name="ypsc")
                        sps = sps4.tile([128, 4, 256], F32, name="spsc")
                        for c in range(4):
                            nc.tensor.matmul(
                                sps[:, c, :],
                                ckT[kb][ko:ko + 64, c * 128:(c + 1) * 128],
                                cqT[kb][ko:ko + 64, qb * 256:(qb + 1) * 256],
                                start=True, stop=True)
                        p_t = pp4.tile([128, 4, 256], FP8, name="Pc")
                        nc.scalar.activation(p_t, sps, AF.Exp, bias=neg2)
                        for qh in range(2):
                            for j2 in range(2):
                                nc.tensor.matmul(
                                    y_ps[:, qh, :],
                                    p_t[:, j2 * 2:(j2 + 1) * 2,
                                        qh * 128:(qh + 1) * 128],
                                    cvP[j2][:, :, h, :],
                                    start=(j2 == 0), stop=(j2 == 1),
                                    perf_mode=DR)
                        for qh in range(2):
                            with nc.allow_low_precision(reason="softmax denom"):
                                nc.gpsimd.tensor_scalar(
                                    ytm[qh][:, h * HD:(h + 1) * HD],
                                    y_ps[:, qh, 0:HD], y_ps[:, qh, HD:HD + 1],
                                    None, OP.divide)
                    pending = (lambda qb=qb, yT8=yT8, ytm=ytm:
                               tail4(qb, yT8, ytm))
                pending()
        pX1.release()
        pC.release()
        w4pre.release()

        # ===== Phase 5: LN2 (feature-major) + MLP + out ===================
        with tc.tile_pool(name="w5", bufs=1) as w5, \
             tc.tile_pool(name="p5a", bufs=1) as p5a, \
             tc.tile_pool(name="p5b", bufs=3) as p5b, \
             tc.tile_pool(name="h0p", bufs=1) as h0p, \
             tc.tile_pool(name="h1p", bufs=1) as h1p, \
             tc.tile_pool(name="oTp", bufs=1) as oTp:
            mw1_sb = [w5.tile([128, 2, HID], FP8, name=f"mw1_{j}")
                      for j in range(3)]
            for j in range(3):
                nc.sync.dma_start(out=mw1_sb[j],
                                  in_=mw1p[j * 128:(j + 1) * 128, :])
            mb1c = w5.tile([128, HCN], F32, name="mb1c")
            nc.sync.dma_start(out=mb1c,
                              in_=mb1.ap().rearrange("(c p) -> p c", p=128))
            mw2_sb = [w5.tile([128, 2, D], FP8, name=f"mw2_{j}")
                      for j in range(12)]
            for j in range(12):
                nc.sync.dma_start(out=mw2_sb[j], in_=mw2p[j * 128:(j + 1) * 128, :])
            h0T8 = [h0p.tile([128, 2, TQ], FP8, name=f"h0T8{j}")
                    for j in range(3)]
            # LN2 stats via ones-matmul partition reduction
            with tc.tile_pool(name="p5st", bufs=1, space="PSUM") as p5st, \
                 tc.tile_pool(name="p5bc", bufs=1, space="PSUM") as p5bc:
                s1 = p5st.tile([1, TQ], F32, name="s1")
                s2 = p5st.tile([1, TQ], F32, name="s2")
                for blk2 in range(2):
                    sl = slice(blk2 * 512, (blk2 + 1) * 512)
                    for dc in range(DC):
                        nc.tensor.matmul(s1[0:1, sl], ones1b, x2Tb[dc][:, sl],
                                         start=(dc == 0), stop=(dc == DC - 1))
                    for dc in range(DC):
                        nc.tensor.matmul(s2[0:1, sl], ones1b, x2sq[dc][:, sl],
                                         start=(dc == 0), stop=(dc == DC - 1))
                mu_n = p5a.tile([1, TQ], F32, name="mu_n")
                nc.vector.tensor_scalar(mu_n, s1, -1.0 / D, None, OP.mult)
                msq = p5a.tile([1, TQ], F32, name="msq")
                nc.vector.tensor_scalar(msq, s2, 1.0 / D, None, OP.mult)
                mu2 = p5a.tile([1, TQ], F32, name="mu2")
                nc.vector.tensor_mul(mu2, mu_n, mu_n)
                var = p5a.tile([1, TQ], F32, name="var")
                nc.vector.tensor_sub(var, msq, mu2)
                # std16 = sqrt((var+eps)/256) = std/16 ; a = 1/std16 = 16*rstd
                std16 = p5a.tile([1, TQ], F32, name="std16")
                nc.scalar.activation(std16, var, AF.Sqrt, bias=eps256,
                                     scale=1.0 / 256.0)
                a_f = p5a.tile([1, TQ], F32, name="a_f")
                nc.vector.reciprocal(a_f, std16)
                a_b = p5a.tile([1, TQ], BF16, name="a_b")
                nc.vector.tensor_copy(a_b, a_f)
                c_b = p5a.tile([1, TQ], BF16, name="c_b")
                nc.vector.tensor_mul(c_b, mu_n, a_f)
                a_bc = p5bc.tile([128, TQ], F32, name="a_bc")
                c_bc = p5bc.tile([128, TQ], F32, name="c_bc")
                for blk2 in range(2):
                    sl = slice(blk2 * 512, (blk2 + 1) * 512)
                    nc.tensor.matmul(a_bc[:, sl], onesrow, a_b[0:1, sl],
                                     start=True, stop=True)
                    nc.tensor.matmul(c_bc[:, sl], onesrow, c_b[0:1, sl],
                                     start=True, stop=True)
                a_sb = p5a.tile([128, TQ], BF16, name="a_sb")
                nc.vector.tensor_copy(a_sb, a_bc)
                c_sb = p5a.tile([128, TQ], BF16, name="c_sb")
                nc.gpsimd.tensor_copy(c_sb, c_bc)
            for dc in range(DC):
                tmp = p5b.tile([128, TQ], BF16, name="h0tmp")
                nc.vector.tensor_mul(tmp, x2Tb[dc], a_sb)
                eng = nc.vector if dc % 2 == 0 else nc.gpsimd
                eng.tensor_tensor(
                    h0T8[dc // 2][:, dc % 2, :], tmp, c_sb, OP.add)
            # h1 = gelu((mw1^T h0 + mb1)) -> fp8
            h1T8 = [h1p.tile([128, 2, TQ], FP8, name=f"h1T8{j}")
                    for j in range(12)]
            x2r = [oTp.tile([128, D], BF16, name=f"x2r{tt}")
                   for tt in range(8)]
            mb2_bc = bias_bc(mb2x, "mb2_bc", w5)
            # x2 transpose-back to token-major (overlaps the LN2 stats
            # chain); residual + mb2 pre-added (mb2_bc is mb2*SW2)
            with tc.tile_pool(name="p5tp", bufs=2, space="PSUM") as p5tp:
                for tt in range(8):
                    pt = p5tp.tile([128, D], BF16, name="x2tT")
                    for dc in range(DC):
                        nc.tensor.transpose(
                            pt[:, dc * 128:(dc + 1) * 128],
                            x2Tb[dc][:, tt * 128:(tt + 1) * 128], identb)
                    nc.gpsimd.scalar_tensor_tensor(
                        x2r[tt], mb2_bc, 1.0 / SW2, pt, OP.mult, OP.add)
            with tc.tile_pool(name="p5m1", bufs=2, space="PSUM") as p5m1, \
                 tc.tile_pool(name="p5m2", bufs=2, space="PSUM") as p5m2, \
                 tc.tile_pool(name="p5o", bufs=3) as p5o:
                for hc in range(HCN):
                    pp = p5m1.tile([128, TQ], F32, name="h1pp")
                    for blk2 in range(2):
                        for half in range(2):
                            sl = slice(blk2 * 512 + half * 256,
                                       blk2 * 512 + (half + 1) * 256)
                            for j in range(3):
                                nc.tensor.matmul(
                                    pp[:, sl],
                                    mw1_sb[j][:, :, hc * 128:(hc + 1) * 128],
                                    h0T8[j][:, :, sl],
                                    start=(j == 0), stop=(j == 2),
                                    perf_mode=DR)
                    nc.scalar.activation(
                        h1T8[hc // 2][:, hc % 2, :], pp, AF.Gelu,
                        bias=mb1c[:, hc:hc + 1], scale=ISXW)
                # h2 token-major: out[tok, feat] interleaves with the gelus
                for tt in range(8):
                    pp = p5m2.tile([128, 4, 256], F32, name="h2pp")
                    # rotate the contraction order so tiles close staggered
                    # instead of all gating on the final gelu
                    jord = [(tt * 3 + i) % 12 for i in range(12)]
                    for i, j in enumerate(jord):
                        for qf in range(4):
                            nc.tensor.matmul(
                                pp[:, qf, 0:192],
                                h1T8[j][:, :, tt * 128:(tt + 1) * 128],
                                mw2_sb[j][:, :, qf * 192:(qf + 1) * 192],
                                start=(i == 0 and qf % 2 == 0),
                                stop=(i == 11 and qf % 2 == 1),
                                perf_mode=DR)
                    o_sb = p5o.tile([128, 4, 192], BF16, name="o_sb")
                    nc.vector.scalar_tensor_tensor(
                        o_sb, pp[:, :, 0:192], 1.0 / SW2,
                        x2r[tt].rearrange("p (a b) -> p a b", a=4),
                        OP.mult, OP.add)
                    nc.sync.dma_start(
                        out=out_own[tt * 128:(tt + 1) * 128, :],
                        in_=o_sb[:, :, :])
        pX2.release()
        singles.release()

    nc.compile()
    return nc


def _get_nc():
    if "nc" not in _CACHE:
        _CACHE["nc"] = _build()
    return _CACHE["nc"]


def _pack2(w, scale):
    """[d_in, d_out] -> [d_in//256*128, 2*d_out] fp8 DoubleRow pair layout."""
    w = np.asarray(w, np.float32)
    d_in, d_out = w.shape
    nj = d_in // 256
    out = np.empty((nj * 128, 2 * d_out), np.float32)
    for j in range(nj):
        out[j * 128:(j + 1) * 128, :d_out] = w[j * 256:j * 256 + 128, :]
        out[j * 128:(j + 1) * 128, d_out:] = w[j * 256 + 128:j * 256 + 256, :]
    out = np.clip(out * scale, -224.0, 224.0)
    return np.ascontiguousarray(out.astype(f8))


def _make_in_maps(inputs):
    x = np.asarray(inputs["x"], np.float32)
    enc = np.asarray(inputs["encoder_hidden_states"], np.float32)
    scale = np.float32(1.0 / np.sqrt(HD))

    f32 = lambda a: np.ascontiguousarray(np.asarray(a, np.float32))
    g1 = np.asarray(inputs["ln1_g"], np.float64)
    b1 = np.asarray(inputs["ln1_b"], np.float64)
    g2 = np.asarray(inputs["ln2_g"], np.float64)
    b2 = np.asarray(inputs["ln2_b"], np.float64)
    sWq = np.asarray(inputs["sWq"], np.float64)
    sWk = np.asarray(inputs["sWk"], np.float64)
    sWv = np.asarray(inputs["sWv"], np.float64)
    mW1 = np.asarray(inputs["mW1"], np.float64)

    shared = dict(
        wqp=_pack2(g1[:, None] * sWq * scale, SW),
        bq=f32((b1 @ sWq + np.asarray(inputs["sbq"], np.float64)) * scale),
        wkp=_pack2(g1[:, None] * sWk, SW),
        bk=f32(b1 @ sWk + np.asarray(inputs["sbk"], np.float64)),
        wvp=_pack2(g1[:, None] * sWv, SW),
        bv=f32(b1 @ sWv + np.asarray(inputs["sbv"], np.float64)),
        wop=_pack2(np.asarray(inputs["sWo"]), SW),
        cwqp=_pack2(np.asarray(inputs["cWq"], np.float64) * scale, SW),
        cbq=f32(np.asarray(inputs["cbq"], np.float64) * scale),
        cwkp=_pack2(np.asarray(inputs["cWk"]), SW),
        cbk=f32(inputs["cbk"]),
        cwvp=_pack2(np.asarray(inputs["cWv"]), SW),
        cbv=f32(inputs["cbv"]),
        cwop=_pack2(np.asarray(inputs["cWo"]), SW),
        mw1p=_pack2(g2[:, None] * mW1, SW),
        mb1=f32(np.asarray(inputs["mb1"], np.float64) + b2 @ mW1),
        mw2p=_pack2(np.asarray(inputs["mW2"]), SW2),
        mb2x=f32(np.asarray(inputs["mb2"], np.float64) * SW2),
        resg=f32(inputs["ln1_g"]),
        resb=f32(b1 + np.asarray(inputs["sbo"], np.float64)
                 + np.asarray(inputs["cbo"], np.float64)),
    )
    kk = np.arange(128)[:, None]
    jq = np.arange(1024)[None, :]
    in_maps = []
    for c in range(8):
        b, p = c // 2, c % 2
        m = dict(shared)
        xb = x[b]
        if p == 1:
            xb = xb.reshape(T // 2, 2, D)[:, ::-1, :].reshape(T, D)
        m["x_full"] = np.ascontiguousarray(xb.astype(ml_dtypes.bfloat16))
        m["encp"] = _pack2(enc[b].T, SE)
        # key row k of a 128-chunk holds global key 128*j + kg(k)
        if p == 0:
            kg = kk
        else:
            kg = kk + 1 - 2 * (kk % 2)
        valid = (2 * (jq % 256) + p) >= (128 * (jq // 256) + kg)
        m["maskq"] = np.ascontiguousarray(
            np.where(valid, 0.0, -30000.0).astype(ml_dtypes.bfloat16))
        in_maps.append(m)
    return in_maps


def kernel(**inputs):
    in_maps = _make_in_maps(inputs)
    nc = _get_nc()
    res = run_bass_kernel_spmd(nc, in_maps, core_ids=list(range(8)))
    out = np.empty((B, T, NINP), np.float32)
    for c in range(8):
        b, p = c // 2, c % 2
        out[b, p::2] = np.asarray(res.results[c]["out_own"], np.float32)
    return out
